# revision 6
# baseline (speedup 1.0000x reference)
"""CRF-RNN (dense Gaussian CRF mean-field) Trainium2 kernel, 8 NeuronCores.

Strategy
--------
N = 8*32*32 = 8192 voxels, L = 21 labels, 5 mean-field iterations.
- Column sharding: core r owns z-slice r (1024 voxels = its output columns).
- Bilateral kernel: each core builds U[:, own] = exp(feat.feat^T - |feat_r|^2/2)
  on device (fp16, SBUF-resident, 16MB). The per-column factor exp(-|feat_c|^2/2)
  cancels against the normalization, so it is never materialized. The
  normalizer is obtained for free as a 22nd "ones" row in the iter-1 matmul.
- Spatial kernel is exactly separable (grid tensor product):
  q@K_s[:, cols_r] = (sum_z Gz[z,r] q[:,z]) @ (Gy x Gx). The z-mix commutes with
  the yx-filter, norm_s is folded into Kyx columns + the z-mix coefficients.
- Per-iteration cross-core exchange of each core's q block ([128,168] fp16)
  via remote_dma_broadcast (SBUF->SBUF, XOR-relative dests). Slot d on core r
  holds core (r XOR d)'s block; K_b rows are host-permuted per core to match,
  so the SPMD program is identical on every core.
- Softmax runs voxel-major (labels on the free dim) - no cross-partition ops.
"""

import numpy as np

ALPHA, BETA, GAMMA = 160.0, 3.0, 3.0
NUM_ITER = 5
L, D, H, W = 21, 8, 32, 32
NC = 8
NYX = H * W            # 1024
N = D * NYX            # 8192
NT = NYX // 128        # 8 chunks per slice
FB = NT * L            # 168  free width of one q block
LW1 = 33               # iter-1 slot entry width: labels 0..20, ones at 32 (aligned)
FB1 = NT * LW1         # iter-1 block width
NTILE = N // 128       # 64 row tiles of the bilateral kernel

_CACHE = {}


def _build_nc():
    import concourse.bass as bass
    import concourse.bacc as bacc
    import concourse.mybir as mybir
    import concourse.tile as tile
    import concourse.tile_utils as tile_utils

    # cayman has 208KB/partition usable; the default cap is stale at 192KB
    try:
        tile_utils.max_sbuf_usage = 204 * 1024
    except Exception:
        pass

    f32 = mybir.dt.float32
    f16 = mybir.dt.float16
    AF = mybir.ActivationFunctionType
    OP = mybir.AluOpType

    nc = bacc.Bacc(None, target_bir_lowering=False, num_devices=NC)

    # ---- DRAM I/O ----
    featr_d = nc.declare_dram_parameter("featr", [6, N], f32, isOutput=False)
    featc_d = nc.declare_dram_parameter("featc", [6, NYX], f32, isOutput=False)
    sqh_d = nc.declare_dram_parameter("sqh", [128, NTILE], f32, isOutput=False)
    kyx_d = nc.declare_dram_parameter("kyx", [128, NT * NYX], f16, isOutput=False)
    unary_d = nc.declare_dram_parameter("unaryt", [128, NC * FB], f32, isOutput=False)
    zco_d = nc.declare_dram_parameter("zcoef", [128, NC], f32, isOutput=False)
    wst_d = nc.declare_dram_parameter("wst", [L, L], f32, isOutput=False)
    wbt_d = nc.declare_dram_parameter("wbt", [L, L], f32, isOutput=False)
    ct_d = nc.declare_dram_parameter("ct", [L, L], f32, isOutput=False)
    ones_d = nc.declare_dram_parameter("ones1", [1, L], f32, isOutput=False)
    out_d = nc.declare_dram_parameter("out", [128, FB], f32, isOutput=True)

    # manual cross-core semaphores
    xch = [nc.alloc_semaphore(f"xch_sem_{j}") for j in range(NUM_ITER - 1)]
    lsem = nc.alloc_semaphore("rdma_local_sem")
    psem = nc.alloc_semaphore("rdma_prep_sem")

    with tile.TileContext(nc) as tc:
        with (
            tc.tile_pool(name="persist", bufs=1) as pp,
            tc.tile_pool(name="stream", bufs=2) as sp,
            tc.tile_pool(name="epi", bufs=3) as ep,
            tc.tile_pool(name="work", bufs=2) as wp,
            tc.tile_pool(name="ps_bil", bufs=1, space="PSUM") as ps_bil,
        ):
            # ---------------- persistent SBUF ----------------
            sb_kb = pp.tile([128, NTILE * NYX], f16, tag="kb")       # 128KB/p
            sb_kyx = pp.tile([128, NT * NYX], f16, tag="kyx")        # 16KB/p
            sb_unary = pp.tile([128, NC * FB], f32, tag="unary")     # 5.25KB/p
            sb_s1 = pp.tile([128, NC * FB1], f16, tag="s1")          # 2.75KB/p
            sb_slots = [pp.tile([128, NC * FB], f16, tag=f"slots{j % 2}",
                                name=f"sb_slots{j}")
                        for j in range(NUM_ITER - 1)]                # 2 sets ping-pong
            sb_raws = [pp.tile([128, NC * FB], f16, tag=f"raw{j}",
                               name=f"sb_raw{j}")
                       for j in range(2)]                            # ping-pong
            sb_featc = pp.tile([6, NYX], f32, tag="featc")
            sb_sqh = pp.tile([128, NTILE], f32, tag="sqh")
            sb_zco = pp.tile([128, NC], f32, tag="zco")
            sb_wst = pp.tile([L, L], f32, tag="wst")
            sb_wbt = pp.tile([L, L], f32, tag="wbt")
            sb_ct = pp.tile([L, L], f32, tag="ct")
            sb_ones = pp.tile([1, L], f32, tag="ones")
            sb_recipb = pp.tile([L, NYX], f32, tag="recipb")
            sb_nrcp = pp.tile([1, NYX], f32, tag="nrcp")
            sb_exp1 = pp.tile([128, NC * FB], f16, tag="exp1")       # iter-1 exp
            sb_red1 = pp.tile([128, NC * NT], f32, tag="red1")
            sb_rcp1 = pp.tile([128, NC * NT], f32, tag="rcp1")
            sb_out = pp.tile([128, FB], f32, tag="outt")

            # ---------------- input DMAs ----------------
            nc.sync.dma_start(sb_featc[:, :], featc_d[:, :])
            nc.sync.dma_start(sb_sqh[:, :], sqh_d[:, :])
            nc.sync.dma_start(sb_kyx[:, :], kyx_d[:, :])
            nc.sync.dma_start(sb_unary[:, :], unary_d[:, :])
            nc.sync.dma_start(sb_zco[:, :], zco_d[:, :])
            nc.sync.dma_start(sb_wst[:, :], wst_d[:, :])
            nc.sync.dma_start(sb_wbt[:, :], wbt_d[:, :])
            nc.sync.dma_start(sb_ct[:, :], ct_d[:, :])
            nc.sync.dma_start(sb_ones[:, :], ones_d[:, :])

            # ---------------- iter-1 softmax for all 8 slots ----------------
            # q1 = softmax(unary) voxel-major; slots1 carry a ones column (22-wide)
            nc.scalar.activation(sb_exp1[:, :], sb_unary[:, :], AF.Exp)
            un_v = sb_exp1[:, :].rearrange("p (g l) -> p g l", l=L)      # [128, 64, 21]
            nc.vector.tensor_reduce(sb_red1[:, :], un_v, mybir.AxisListType.X, OP.add)
            nc.vector.reciprocal(sb_rcp1[:, :], sb_red1[:, :])
            nc.vector.memset(sb_s1[:, :], 0.0)
            s1_v = sb_s1[:, :].rearrange("p (g l) -> p g l", l=LW1)      # [128, 64, 33]
            nc.vector.tensor_tensor(
                s1_v[:, :, 0:L], un_v,
                sb_rcp1[:, :].broadcast_to([128, NC * NT, L]),
                OP.mult,
            )
            nc.vector.memset(s1_v[:, :, LW1 - 1 : LW1], 1.0)

            # ---------------- fused K_b build + iter-1 bilateral ----------------
            ps1 = ps_bil.tile([LW1, NYX], f32, tag="bil")
            kb_v = sb_kb[:, :].rearrange("p (n c) -> p n c", c=NYX)      # [128, 64, 1024]
            s1_l = sb_s1[:, :].rearrange("p (n l) -> p n l", l=LW1)      # [128, 64, 33]
            with tc.tile_pool(name="ps_g", bufs=2, space="PSUM") as ps_g:
                for mc in range(NT):  # macro chunks of 8 tiles
                    fr = sp.tile([6, NYX], f32, tag="fr")
                    nc.sync.dma_start(fr[:, :], featr_d[:, mc * NYX:(mc + 1) * NYX])
                    for tl in range(NT):
                        dt = mc * NT + tl
                        g = ps_g.tile([128, NYX], f32, tag="g")
                        for h in range(2):
                            nc.tensor.matmul(
                                g[:, h * 512:(h + 1) * 512],
                                fr[:, tl * 128:(tl + 1) * 128],
                                sb_featc[:, h * 512:(h + 1) * 512],
                                start=True, stop=True,
                            )
                        for h in range(2):
                            nc.scalar.activation(
                                kb_v[:, dt, h * 512:(h + 1) * 512],
                                g[:, h * 512:(h + 1) * 512],
                                AF.Exp, bias=sb_sqh[:, dt:dt + 1],
                            )
                        for h in range(2):
                            nc.tensor.matmul(
                                ps1[:, h * 512:(h + 1) * 512],
                                s1_l[:, dt, :],
                                kb_v[:, dt, h * 512:(h + 1) * 512],
                                start=(dt == 0), stop=(dt == NTILE - 1),
                                skip_group_check=True,
                            )

            with tc.tile_pool(name="ps_rest", bufs=1, space="PSUM") as ps_r:
                # ---------------- norm reciprocal + broadcast ----------------
                # DMA: engines cannot address partition base 21, DMA can
                nc.vector.reciprocal(sb_nrcp[:, :], ps1[LW1 - 1:LW1, :])
                ps_nb = ps_r.tile([L, NYX], f32, tag="spat")
                for h in range(2):
                    nc.tensor.matmul(
                        ps_nb[:, h * 512:(h + 1) * 512], sb_ones[:, :],
                        sb_nrcp[:, h * 512:(h + 1) * 512], start=True, stop=True,
                    )
                nc.scalar.copy(sb_recipb[:, :], ps_nb[:, :])

                # ================= iterations =================
                cur_bil = ps1
                slots_l = s1_l
                slots_dt = sb_s1[:, :].rearrange(
                    "p (d t l) -> p d t l", d=NC, l=LW1)

                for it in range(NUM_ITER):
                    last = it == NUM_ITER - 1
                    # ---- bilateral message (iters >= 2; iter 1 fused above) ----
                    if it > 0:
                        cur_bil = ps_bil.tile([LW1, NYX], f32, tag="bil")
                        for d in range(NC):
                            for tl in range(NT):
                                dt = d * NT + tl
                                for h in range(2):
                                    nc.tensor.matmul(
                                        cur_bil[0:L, h * 512:(h + 1) * 512],
                                        slots_l[:, dt, 0:L],
                                        kb_v[:, dt, h * 512:(h + 1) * 512],
                                        start=(dt == 0), stop=(dt == NTILE - 1),
                                        skip_group_check=True,
                                    )
                    mbn = ep.tile([L, NYX], f32, tag="epi")
                    nc.vector.tensor_tensor(
                        mbn[:, :], cur_bil[0:L, :], sb_recipb[:, :], OP.mult)

                    # ---- spatial: z-mix then yx filter ----
                    brt = wp.tile([128, FB], f16, tag="brt")
                    nc.vector.tensor_scalar_mul(
                        brt[:, :], slots_dt[:, 0, :, 0:L], sb_zco[:, 0:1])
                    for d in range(1, NC):
                        nc.vector.scalar_tensor_tensor(
                            brt[:, :], slots_dt[:, d, :, 0:L], sb_zco[:, d:d + 1],
                            brt[:, :], OP.mult, OP.add)
                    brt_v = brt[:, :].rearrange("p (t l) -> p t l", l=L)
                    ps_sp = ps_r.tile([L, NYX], f32, tag="spat")
                    kyx_v = sb_kyx[:, :].rearrange("p (k c) -> p k c", c=NYX)
                    for k in range(NT):
                        for h in range(2):
                            nc.tensor.matmul(
                                ps_sp[:, h * 512:(h + 1) * 512],
                                brt_v[:, k, :],
                                kyx_v[:, k, h * 512:(h + 1) * 512],
                                start=(k == 0), stop=(k == NT - 1),
                                skip_group_check=True,
                            )
                    msn = ep.tile([L, NYX], f32, tag="epi")
                    nc.scalar.copy(msn[:, :], ps_sp[:, :])

                    # ---- label mixing: comb = Ws@Msn + Wb@Mbn ----
                    ps_cb = ps_r.tile([L, NYX], f32, tag="comb")
                    for h in range(2):
                        nc.tensor.matmul(
                            ps_cb[:, h * 512:(h + 1) * 512], sb_wst[:, :],
                            msn[:, h * 512:(h + 1) * 512],
                            start=True, stop=False, skip_group_check=True)
                        nc.tensor.matmul(
                            ps_cb[:, h * 512:(h + 1) * 512], sb_wbt[:, :],
                            mbn[:, h * 512:(h + 1) * 512],
                            start=False, stop=True, skip_group_check=True)
                    comb = ep.tile([L, NYX], f32, tag="epi")
                    nc.scalar.copy(comb[:, :], ps_cb[:, :])

                    # ---- curT = comb^T @ C^T (+ unary), voxel-major ----
                    ps_ct = ps_r.tile([128, FB], f32, tag="curt")
                    for tl in range(NT):
                        nc.tensor.matmul(
                            ps_ct[:, tl * L:(tl + 1) * L],
                            comb[:, tl * 128:(tl + 1) * 128],
                            sb_ct[:, :], start=True, stop=True)
                    sm = wp.tile([128, FB], f32, tag="sum")
                    nc.vector.tensor_tensor(
                        sm[:, :], ps_ct[:, :], sb_unary[:, 0:FB], OP.add)

                    # ---- softmax over labels (free dim) ----
                    ex = wp.tile([128, FB], f32, tag="exp")
                    nc.scalar.activation(ex[:, :], sm[:, :], AF.Exp)
                    ex_v = ex[:, :].rearrange("p (t l) -> p t l", l=L)
                    rd = wp.tile([128, NT], f32, tag="red")
                    nc.vector.tensor_reduce(
                        rd[:, :], ex_v, mybir.AxisListType.X, OP.add)
                    rc = wp.tile([128, NT], f32, tag="rcp")
                    nc.vector.reciprocal(rc[:, :], rd[:, :])
                    if last:
                        nc.vector.tensor_tensor(
                            sb_out[:, :].rearrange("p (t l) -> p t l", l=L),
                            ex_v, rc[:, :].broadcast_to([128, NT, L]), OP.mult)
                        nc.sync.dma_start(out_d[:, :], sb_out[:, :])
                    else:
                        nxt = sb_slots[it]
                        nxt_dt = nxt[:, :].rearrange(
                            "p (d t l) -> p d t l", d=NC, l=L)
                        nc.vector.tensor_tensor(
                            nxt_dt[:, 0, :, :], ex_v,
                            rc[:, :].broadcast_to([128, NT, L]), OP.mult)

                        # ---- exchange: send own block to the 7 peers ----
                        raw = sb_raws[it % 2]
                        with tc.tile_critical():
                            if it > 0:
                                nc.gpsimd.wait_ge(lsem, 16 * (NC - 1) * it)
                            for d in range(1, NC):
                                rdests = [None] * NC
                                rdests[d] = (0, d)
                                nc.gpsimd.remote_dma_broadcast(
                                    raw[:, d * FB:(d + 1) * FB],
                                    nxt[:, 0:FB],
                                    remote_sem=xch[it], local_sem=lsem,
                                    rdests=rdests,
                                ).then_inc(psem, 1)
                            nc.gpsimd.wait_ge(psem, (NC - 1) * (it + 1))
                            nc.gpsimd.trigger_dma(NC - 1)

                        # ---- receive: wait for 7 blocks, copy into slots ----
                        with tc.tile_critical():
                            nc.vector.wait_ge(xch[it], 2 * (NC - 1))
                            for d in range(1, NC):
                                nc.vector.tensor_copy(
                                    nxt[:, d * FB:(d + 1) * FB],
                                    raw[:, d * FB:(d + 1) * FB])

                        slots_l = nxt[:, :].rearrange("p (n l) -> p n l", l=L)
                        slots_dt = nxt_dt
    nc.compile()
    return nc


def _host_prep(image, logits):
    """Per-core input dicts. Returns list of 8 dicts."""
    img = np.asarray(image, dtype=np.float32)[0]      # [3, D, H, W]
    lg = np.asarray(logits, dtype=np.float32)[0]      # [L, D, H, W]

    zz, yy, xx = np.meshgrid(
        np.arange(D), np.arange(H), np.arange(W), indexing="ij")
    pos = np.stack([zz, yy, xx], -1).reshape(N, 3).astype(np.float32)
    rgb = img.reshape(3, N).T
    feat = np.concatenate([pos / ALPHA, rgb / BETA], axis=1)   # [N, 6]
    sq = np.sum(feat * feat, axis=1)                           # [N]

    r1 = np.arange(D, dtype=np.float32)
    Gz = np.exp(-0.5 * ((r1[:, None] - r1[None, :]) / GAMMA) ** 2)
    r2 = np.arange(H, dtype=np.float32)
    Gy = np.exp(-0.5 * ((r2[:, None] - r2[None, :]) / GAMMA) ** 2)
    Kyx = np.kron(Gy, Gy).astype(np.float32)            # H == W so Gy == Gx
    nyx = Kyx.sum(axis=0)
    Kyx_n = (Kyx / nyx[None, :]).astype(np.float16)     # [1024, 1024]
    czsum = Gz.sum(axis=0)

    unary = lg.reshape(L, N)                            # [L, N]
    # voxel-major per slice: blkT[s][p, t*L + l] = unary[l, s*NYX + t*128 + p]
    blkT = unary.reshape(L, D, NT, 128).transpose(3, 1, 2, 0)   # [128, D, NT, L]

    kyx_in = np.ascontiguousarray(
        Kyx_n.reshape(NT, 128, NYX).transpose(1, 0, 2).reshape(128, NT * NYX))

    maps = []
    for r in range(NC):
        perm_sl = [r ^ d for d in range(NC)]
        perm = np.concatenate(
            [np.arange(s * NYX, (s + 1) * NYX) for s in perm_sl])
        featr = np.ascontiguousarray(feat[perm].T)               # [6, N]
        featc = np.ascontiguousarray(feat[r * NYX:(r + 1) * NYX].T)
        sqh = np.ascontiguousarray(
            (-0.5 * sq[perm]).reshape(NTILE, 128).T)             # [128, 64]
        un = np.ascontiguousarray(
            blkT[:, perm_sl, :, :].reshape(128, NC * FB))        # [128, 1344]
        zco = np.tile(
            (Gz[perm_sl, r] / czsum[r]).astype(np.float32), (128, 1))
        maps.append({
            "featr": featr,
            "featc": featc,
            "sqh": sqh,
            "kyx": kyx_in,
            "unaryt": un,
            "zcoef": np.ascontiguousarray(zco),
        })
    return maps


def kernel(image, logits, spatial_ker_weights, bilateral_ker_weights,
           compatibility_matrix):
    from concourse.bass_utils import run_bass_kernel_spmd

    if "nc" not in _CACHE:
        _CACHE["nc"] = _build_nc()
    nc = _CACHE["nc"]

    maps = _host_prep(image, logits)
    wst = np.ascontiguousarray(
        np.asarray(spatial_ker_weights, np.float32).T)
    wbt = np.ascontiguousarray(
        np.asarray(bilateral_ker_weights, np.float32).T)
    ct = np.ascontiguousarray(
        np.asarray(compatibility_matrix, np.float32).T)
    ones1 = np.ones((1, L), np.float32)
    for m in maps:
        m["wst"] = wst
        m["wbt"] = wbt
        m["ct"] = ct
        m["ones1"] = ones1

    res = run_bass_kernel_spmd(nc, maps, core_ids=list(range(NC)))

    out = np.empty((L, D, H, W), dtype=np.float32)
    for r in range(NC):
        blk = res.results[r]["out"]                   # [128, 168]
        # blk[p, t*L + l] = q[l, slice r, t*128 + p]
        out[:, r] = blk.reshape(128, NT, L).transpose(2, 1, 0).reshape(L, H, W)
    return out[None]


# revision 7
# speedup vs baseline: 22.2739x; 22.2739x over previous
"""CRF-RNN (dense Gaussian CRF mean-field) Trainium2 kernel, 8 NeuronCores.

Strategy
--------
N = 8*32*32 = 8192 voxels, L = 21 labels, 5 mean-field iterations.
- Column sharding: core r owns z-slice r (1024 voxels = its output columns).
- Bilateral kernel: each core builds U[:, own] = exp(feat.feat^T - |feat_r|^2/2)
  on device (fp16, SBUF-resident, 16MB). The per-column factor exp(-|feat_c|^2/2)
  cancels against the normalization, so it is never materialized. The
  normalizer is obtained for free as an extra "ones" row in the iter-1 matmul
  (placed at partition 32 so engines can address it).
- Spatial kernel is exactly separable (grid tensor product):
  q@K_s[:, cols_r] = (sum_z Gz[z,r] q[:,z]) @ (Gy x Gx). The z-mix commutes with
  the yx-filter; norm_s is folded into Kyx columns + the z-mix coefficients.
- Per-iteration cross-core exchange of each core's q block ([128,168] fp16)
  via AllGather through DRAM bounce buffers (ncfw collective).
- Softmax runs voxel-major (labels on the free dim) - no cross-partition ops.
"""

import numpy as np

ALPHA, BETA, GAMMA = 160.0, 3.0, 3.0
NUM_ITER = 5
L, D, H, W = 21, 8, 32, 32
NC = 8
NYX = H * W            # 1024
N = D * NYX            # 8192
NT = NYX // 128        # 8 chunks per slice
FB = NT * L            # 168  free width of one q block
LW1 = 33               # iter-1 slot entry width: labels 0..20, ones at 32
FB1 = NT * LW1
NTILE = N // 128       # 64 row tiles of the bilateral kernel

_CACHE = {}


def _build_nc():
    import concourse.bass as bass
    import concourse.bacc as bacc
    import concourse.mybir as mybir
    import concourse.tile as tile
    import concourse.tile_utils as tile_utils

    # cayman has 208KB/partition usable; the default cap is stale at 192KB
    try:
        tile_utils.max_sbuf_usage = 204 * 1024
    except Exception:
        pass

    f32 = mybir.dt.float32
    f16 = mybir.dt.float16
    AF = mybir.ActivationFunctionType
    OP = mybir.AluOpType

    nc = bacc.Bacc(None, target_bir_lowering=False, num_devices=NC)

    # ---- DRAM I/O (fp16 features; global voxel order everywhere) ----
    featr_d = nc.declare_dram_parameter("featr", [6, N], f16, isOutput=False)
    featc_d = nc.declare_dram_parameter("featc", [6, NYX], f16, isOutput=False)
    sqh_d = nc.declare_dram_parameter("sqh", [128, NTILE], f32, isOutput=False)
    kyx_d = nc.declare_dram_parameter("kyx", [128, NT * NYX], f16, isOutput=False)
    unary_d = nc.declare_dram_parameter("unaryt", [128, NC * FB], f32, isOutput=False)
    unown_d = nc.declare_dram_parameter("unown", [128, FB], f32, isOutput=False)
    zco_d = nc.declare_dram_parameter("zcoef", [128, NC], f32, isOutput=False)
    wst_d = nc.declare_dram_parameter("wst", [L, L], f32, isOutput=False)
    wbt_d = nc.declare_dram_parameter("wbt", [L, L], f32, isOutput=False)
    ct_d = nc.declare_dram_parameter("ct", [L, L], f32, isOutput=False)
    ones_d = nc.declare_dram_parameter("ones1", [1, L], f32, isOutput=False)
    out_d = nc.declare_dram_parameter("out", [128, FB], f32, isOutput=True)

    with tile.TileContext(nc) as tc:
        with (
            tc.tile_pool(name="persist", bufs=1) as pp,
            tc.tile_pool(name="stream", bufs=2) as sp,
            tc.tile_pool(name="epi", bufs=3) as ep,
            tc.tile_pool(name="work", bufs=2) as wp,
            tc.tile_pool(name="dram", bufs=2, space="DRAM") as dp,
            tc.tile_pool(name="ps_bil", bufs=1, space="PSUM") as ps_bil,
        ):
            # ---------------- persistent SBUF ----------------
            sb_kb = pp.tile([128, NTILE * NYX], f16, tag="kb")       # 128KB/p
            sb_kyx = pp.tile([128, NT * NYX], f16, tag="kyx")        # 16KB/p
            sb_unary = pp.tile([128, NC * FB], f32, tag="unary")     # 5.25KB/p
            sb_unown = pp.tile([128, FB], f32, tag="unown")
            sb_s1 = pp.tile([128, NC * FB1], f16, tag="s1")          # 4.1KB/p
            sb_slots = [pp.tile([128, NC * FB], f16, tag=f"slots{j % 2}",
                                name=f"sb_slots{j}")
                        for j in range(NUM_ITER - 1)]                # ping-pong
            sb_featc = pp.tile([6, NYX], f16, tag="featc")
            sb_sqh = pp.tile([128, NTILE], f32, tag="sqh")
            sb_zco = pp.tile([128, NC], f32, tag="zco")
            sb_wst = pp.tile([L, L], f32, tag="wst")
            sb_wbt = pp.tile([L, L], f32, tag="wbt")
            sb_ct = pp.tile([L, L], f32, tag="ct")
            sb_ones = pp.tile([1, L], f32, tag="ones")
            sb_recipb = pp.tile([L, NYX], f32, tag="recipb")
            sb_nrcp = pp.tile([1, NYX], f32, tag="nrcp")
            sb_exp1 = pp.tile([128, NC * FB], f16, tag="exp1")       # iter-1 exp
            sb_red1 = pp.tile([128, NC * NT], f32, tag="red1")
            sb_rcp1 = pp.tile([128, NC * NT], f32, tag="rcp1")
            sb_out = pp.tile([128, FB], f32, tag="outt")

            # ---------------- input DMAs ----------------
            nc.sync.dma_start(sb_featc[:, :], featc_d[:, :])
            nc.sync.dma_start(sb_sqh[:, :], sqh_d[:, :])
            nc.sync.dma_start(sb_kyx[:, :], kyx_d[:, :])
            nc.sync.dma_start(sb_unary[:, :], unary_d[:, :])
            nc.sync.dma_start(sb_unown[:, :], unown_d[:, :])
            nc.sync.dma_start(sb_zco[:, :], zco_d[:, :])
            nc.sync.dma_start(sb_wst[:, :], wst_d[:, :])
            nc.sync.dma_start(sb_wbt[:, :], wbt_d[:, :])
            nc.sync.dma_start(sb_ct[:, :], ct_d[:, :])
            nc.sync.dma_start(sb_ones[:, :], ones_d[:, :])

            # ---------------- iter-1 softmax for all 8 slices ----------------
            nc.scalar.activation(sb_exp1[:, :], sb_unary[:, :], AF.Exp)
            un_v = sb_exp1[:, :].rearrange("p (g l) -> p g l", l=L)      # [128, 64, 21]
            nc.vector.tensor_reduce(sb_red1[:, :], un_v, mybir.AxisListType.X, OP.add)
            nc.vector.reciprocal(sb_rcp1[:, :], sb_red1[:, :])
            nc.vector.memset(sb_s1[:, :], 0.0)
            s1_v = sb_s1[:, :].rearrange("p (g l) -> p g l", l=LW1)      # [128, 64, 33]
            nc.vector.tensor_tensor(
                s1_v[:, :, 0:L], un_v,
                sb_rcp1[:, :].broadcast_to([128, NC * NT, L]),
                OP.mult,
            )
            nc.vector.memset(s1_v[:, :, LW1 - 1:LW1], 1.0)

            # ---------------- fused K_b build + iter-1 bilateral ----------------
            ps1 = ps_bil.tile([LW1, NYX], f32, tag="bil")
            kb_v = sb_kb[:, :].rearrange("p (n c) -> p n c", c=NYX)      # [128, 64, 1024]
            s1_l = sb_s1[:, :].rearrange("p (n l) -> p n l", l=LW1)      # [128, 64, 33]
            with tc.tile_pool(name="ps_g", bufs=2, space="PSUM") as ps_g:
                for mc in range(NT):  # macro chunks of 8 tiles
                    fr = sp.tile([6, NYX], f16, tag="fr")
                    nc.sync.dma_start(fr[:, :], featr_d[:, mc * NYX:(mc + 1) * NYX])
                    for tl in range(NT):
                        dt = mc * NT + tl
                        g = ps_g.tile([128, NYX], f32, tag="g")
                        for h in range(2):
                            nc.tensor.matmul(
                                g[:, h * 512:(h + 1) * 512],
                                fr[:, tl * 128:(tl + 1) * 128],
                                sb_featc[:, h * 512:(h + 1) * 512],
                                start=True, stop=True,
                            )
                        for h in range(2):
                            nc.scalar.activation(
                                kb_v[:, dt, h * 512:(h + 1) * 512],
                                g[:, h * 512:(h + 1) * 512],
                                AF.Exp, bias=sb_sqh[:, dt:dt + 1],
                            )
                        for h in range(2):
                            nc.tensor.matmul(
                                ps1[:, h * 512:(h + 1) * 512],
                                s1_l[:, dt, :],
                                kb_v[:, dt, h * 512:(h + 1) * 512],
                                start=(dt == 0), stop=(dt == NTILE - 1),
                                skip_group_check=True,
                            )

            with tc.tile_pool(name="ps_rest", bufs=1, space="PSUM") as ps_r:
                # ---------------- bilateral norm reciprocal + broadcast ----------------
                nc.vector.reciprocal(sb_nrcp[:, :], ps1[LW1 - 1:LW1, :])
                ps_nb = ps_r.tile([L, NYX], f32, tag="spat")
                for h in range(2):
                    nc.tensor.matmul(
                        ps_nb[:, h * 512:(h + 1) * 512], sb_ones[:, :],
                        sb_nrcp[:, h * 512:(h + 1) * 512], start=True, stop=True,
                    )
                nc.scalar.copy(sb_recipb[:, :], ps_nb[:, :])

                # ================= iterations =================
                cur_bil = ps1
                slots_l = s1_l
                slots_dt = sb_s1[:, :].rearrange(
                    "p (d t l) -> p d t l", d=NC, l=LW1)
                lw = LW1

                for it in range(NUM_ITER):
                    last = it == NUM_ITER - 1
                    # ---- bilateral message (iter 1 was fused above) ----
                    if it > 0:
                        cur_bil = ps_bil.tile([LW1, NYX], f32, tag="bil")
                        for dt in range(NTILE):
                            for h in range(2):
                                nc.tensor.matmul(
                                    cur_bil[0:L, h * 512:(h + 1) * 512],
                                    slots_l[:, dt, 0:L],
                                    kb_v[:, dt, h * 512:(h + 1) * 512],
                                    start=(dt == 0), stop=(dt == NTILE - 1),
                                    skip_group_check=True,
                                )
                    mbn = ep.tile([L, NYX], f32, tag="epi")
                    nc.vector.tensor_tensor(
                        mbn[:, :], cur_bil[0:L, :], sb_recipb[:, :], OP.mult)

                    # ---- spatial: z-mix then yx filter ----
                    brt = wp.tile([128, FB], f16, tag="brt")
                    nc.vector.tensor_scalar_mul(
                        brt[:, :], slots_dt[:, 0, :, 0:L], sb_zco[:, 0:1])
                    for d in range(1, NC):
                        nc.vector.scalar_tensor_tensor(
                            brt[:, :], slots_dt[:, d, :, 0:L], sb_zco[:, d:d + 1],
                            brt[:, :], OP.mult, OP.add)
                    brt_v = brt[:, :].rearrange("p (t l) -> p t l", l=L)
                    ps_sp = ps_r.tile([L, NYX], f32, tag="spat")
                    kyx_v = sb_kyx[:, :].rearrange("p (k c) -> p k c", c=NYX)
                    for k in range(NT):
                        for h in range(2):
                            nc.tensor.matmul(
                                ps_sp[:, h * 512:(h + 1) * 512],
                                brt_v[:, k, :],
                                kyx_v[:, k, h * 512:(h + 1) * 512],
                                start=(k == 0), stop=(k == NT - 1),
                                skip_group_check=True,
                            )
                    msn = ep.tile([L, NYX], f32, tag="epi")
                    nc.scalar.copy(msn[:, :], ps_sp[:, :])

                    # ---- label mixing: comb = Ws@Msn + Wb@Mbn ----
                    ps_cb = ps_r.tile([L, NYX], f32, tag="comb")
                    for h in range(2):
                        nc.tensor.matmul(
                            ps_cb[:, h * 512:(h + 1) * 512], sb_wst[:, :],
                            msn[:, h * 512:(h + 1) * 512],
                            start=True, stop=False, skip_group_check=True)
                        nc.tensor.matmul(
                            ps_cb[:, h * 512:(h + 1) * 512], sb_wbt[:, :],
                            mbn[:, h * 512:(h + 1) * 512],
                            start=False, stop=True, skip_group_check=True)
                    comb = ep.tile([L, NYX], f32, tag="epi")
                    nc.scalar.copy(comb[:, :], ps_cb[:, :])

                    # ---- curT = comb^T @ C^T (+ unary), voxel-major ----
                    ps_ct = ps_r.tile([128, FB], f32, tag="curt")
                    for tl in range(NT):
                        nc.tensor.matmul(
                            ps_ct[:, tl * L:(tl + 1) * L],
                            comb[:, tl * 128:(tl + 1) * 128],
                            sb_ct[:, :], start=True, stop=True)
                    sm = wp.tile([128, FB], f32, tag="sum")
                    nc.vector.tensor_tensor(
                        sm[:, :], ps_ct[:, :], sb_unown[:, :], OP.add)

                    # ---- softmax over labels (free dim) ----
                    ex = wp.tile([128, FB], f32, tag="exp")
                    nc.scalar.activation(ex[:, :], sm[:, :], AF.Exp)
                    ex_v = ex[:, :].rearrange("p (t l) -> p t l", l=L)
                    rd = wp.tile([128, NT], f32, tag="red")
                    nc.vector.tensor_reduce(
                        rd[:, :], ex_v, mybir.AxisListType.X, OP.add)
                    rc = wp.tile([128, NT], f32, tag="rcp")
                    nc.vector.reciprocal(rc[:, :], rd[:, :])
                    if last:
                        nc.vector.tensor_tensor(
                            sb_out[:, :].rearrange("p (t l) -> p t l", l=L),
                            ex_v, rc[:, :].broadcast_to([128, NT, L]), OP.mult)
                        nc.sync.dma_start(out_d[:, :], sb_out[:, :])
                    else:
                        qblk = wp.tile([128, FB], f16, tag="qblk")
                        nc.vector.tensor_tensor(
                            qblk[:, :].rearrange("p (t l) -> p t l", l=L),
                            ex_v, rc[:, :].broadcast_to([128, NT, L]), OP.mult)

                        # ---- exchange: AllGather of the q blocks ----
                        cc_in = dp.tile([128, FB], f16, tag="ccin")
                        cc_out = dp.tile([128 * NC, FB], f16, tag="ccout")
                        nc.sync.dma_start(cc_in[:, :], qblk[:, :])
                        nc.gpsimd.collective_compute(
                            "AllGather",
                            mybir.AluOpType.bypass,
                            replica_groups=[list(range(NC))],
                            ins=[cc_in.opt()],
                            outs=[cc_out.opt()],
                        )
                        nxt = sb_slots[it]
                        nc.sync.dma_start(
                            nxt[:, :].rearrange("p (d f) -> p d f", d=NC),
                            cc_out[:, :].rearrange("(d p) f -> p d f", p=128),
                        )
                        slots_l = nxt[:, :].rearrange("p (n l) -> p n l", l=L)
                        slots_dt = nxt[:, :].rearrange(
                            "p (d t l) -> p d t l", d=NC, l=L)
    nc.compile()
    return nc


def _host_prep(image, logits):
    """Per-core input dicts (global voxel order). Returns list of 8 dicts."""
    img = np.asarray(image, dtype=np.float32)[0]      # [3, D, H, W]
    lg = np.asarray(logits, dtype=np.float32)[0]      # [L, D, H, W]

    zz, yy, xx = np.meshgrid(
        np.arange(D), np.arange(H), np.arange(W), indexing="ij")
    pos = np.stack([zz, yy, xx], -1).reshape(N, 3).astype(np.float32)
    rgb = img.reshape(3, N).T
    feat = np.concatenate([pos / ALPHA, rgb / BETA], axis=1).astype(np.float16)
    featf = feat.astype(np.float32)
    sq = np.sum(featf * featf, axis=1)                # [N], from fp16-rounded

    r1 = np.arange(D, dtype=np.float32)
    Gz = np.exp(-0.5 * ((r1[:, None] - r1[None, :]) / GAMMA) ** 2)
    r2 = np.arange(H, dtype=np.float32)
    Gy = np.exp(-0.5 * ((r2[:, None] - r2[None, :]) / GAMMA) ** 2)
    Kyx = np.kron(Gy, Gy).astype(np.float32)          # H == W so Gy == Gx
    nyx = Kyx.sum(axis=0)
    Kyx_n = (Kyx / nyx[None, :]).astype(np.float16)   # [1024, 1024]
    czsum = Gz.sum(axis=0)

    unary = lg.reshape(L, N)
    # voxel-major: blkT[p, s, t*L + l] = unary[l, s*NYX + t*128 + p]
    blkT = unary.reshape(L, D, NT, 128).transpose(3, 1, 2, 0)  # [128, D, NT, L]
    un = np.ascontiguousarray(blkT.reshape(128, NC * FB))

    featr = np.ascontiguousarray(feat.T)              # [6, N] fp16
    sqh = np.ascontiguousarray((-0.5 * sq).reshape(NTILE, 128).T)  # [128, 64]
    kyx_in = np.ascontiguousarray(
        Kyx_n.reshape(NT, 128, NYX).transpose(1, 0, 2).reshape(128, NT * NYX))

    maps = []
    for r in range(NC):
        featc = np.ascontiguousarray(feat[r * NYX:(r + 1) * NYX].T)
        zco = np.tile((Gz[:, r] / czsum[r]).astype(np.float32), (128, 1))
        unown = np.ascontiguousarray(blkT[:, r].reshape(128, FB))
        maps.append({
            "featr": featr,
            "featc": featc,
            "sqh": sqh,
            "kyx": kyx_in,
            "unaryt": un,
            "unown": unown,
            "zcoef": np.ascontiguousarray(zco),
        })
    return maps


def kernel(image, logits, spatial_ker_weights, bilateral_ker_weights,
           compatibility_matrix):
    from concourse.bass_utils import run_bass_kernel_spmd

    if "nc" not in _CACHE:
        _CACHE["nc"] = _build_nc()
    nc = _CACHE["nc"]

    maps = _host_prep(image, logits)
    wst = np.ascontiguousarray(
        np.asarray(spatial_ker_weights, np.float32).T)
    wbt = np.ascontiguousarray(
        np.asarray(bilateral_ker_weights, np.float32).T)
    ct = np.ascontiguousarray(
        np.asarray(compatibility_matrix, np.float32).T)
    ones1 = np.ones((1, L), np.float32)
    for m in maps:
        m["wst"] = wst
        m["wbt"] = wbt
        m["ct"] = ct
        m["ones1"] = ones1

    res = run_bass_kernel_spmd(nc, maps, core_ids=list(range(NC)))

    out = np.empty((L, D, H, W), dtype=np.float32)
    for r in range(NC):
        blk = res.results[r]["out"]                   # [128, 168]
        out[:, r] = blk.reshape(128, NT, L).transpose(2, 1, 0).reshape(L, H, W)
    return out[None]


# revision 9
# speedup vs baseline: 23.7507x; 1.0663x over previous
"""CRF-RNN (dense Gaussian CRF mean-field) Trainium2 kernel, 8 NeuronCores.

Strategy
--------
N = 8*32*32 = 8192 voxels, L = 21 labels, 5 mean-field iterations.
- Column sharding: core r owns z-slice r (1024 voxels = its output columns).
- Bilateral kernel: each core builds U[:, own] = exp(feat.feat^T - |feat_r|^2/2)
  on device (fp16, SBUF-resident, 16MB). The per-column factor exp(-|feat_c|^2/2)
  cancels against the normalization, so it is never materialized. The
  normalizer is obtained for free as an extra "ones" row in the iter-1 matmul
  (placed at partition 32 so engines can address it).
- Spatial kernel is exactly separable (grid tensor product):
  q@K_s[:, cols_r] = (sum_z Gz[z,r] q[:,z]) @ (Gy x Gx). The z-mix commutes with
  the yx-filter; norm_s is folded into Kyx columns + the z-mix coefficients.
- Per-iteration cross-core exchange of each core's q block ([128,168] fp16)
  via AllGather through DRAM bounce buffers (ncfw collective).
- Softmax runs voxel-major (labels on the free dim) - no cross-partition ops.
"""

import numpy as np

ALPHA, BETA, GAMMA = 160.0, 3.0, 3.0
NUM_ITER = 5
L, D, H, W = 21, 8, 32, 32
NC = 8
NYX = H * W            # 1024
N = D * NYX            # 8192
NT = NYX // 128        # 8 chunks per slice
FB = NT * L            # 168  free width of one q block
LW1 = 33               # iter-1 slot entry width: labels 0..20, ones at 32
FB1 = NT * LW1
NTILE = N // 128       # 64 row tiles of the bilateral kernel

_CACHE = {}


def _build_nc():
    import concourse.bass as bass
    import concourse.bacc as bacc
    import concourse.mybir as mybir
    import concourse.tile as tile
    import concourse.tile_utils as tile_utils

    # cayman has 208KB/partition usable; the default cap is stale at 192KB
    try:
        tile_utils.max_sbuf_usage = 204 * 1024
    except Exception:
        pass

    f32 = mybir.dt.float32
    f16 = mybir.dt.float16
    AF = mybir.ActivationFunctionType
    OP = mybir.AluOpType

    nc = bacc.Bacc(None, target_bir_lowering=False, num_devices=NC)

    # ---- DRAM I/O (fp16 features; global voxel order everywhere) ----
    featr_d = nc.declare_dram_parameter("featr", [6, N], f16, isOutput=False)
    featc_d = nc.declare_dram_parameter("featc", [6, NYX], f16, isOutput=False)
    sqh_d = nc.declare_dram_parameter("sqh", [128, NTILE], f32, isOutput=False)
    kyx_d = nc.declare_dram_parameter("kyx", [128, NT * NYX], f16, isOutput=False)
    unary_d = nc.declare_dram_parameter("unaryt", [128, NC * FB], f32, isOutput=False)
    unown_d = nc.declare_dram_parameter("unown", [128, FB], f32, isOutput=False)
    zco_d = nc.declare_dram_parameter("zcoef", [128, NC], f32, isOutput=False)
    wst_d = nc.declare_dram_parameter("wst", [L, L], f32, isOutput=False)
    wbt_d = nc.declare_dram_parameter("wbt", [L, L], f32, isOutput=False)
    ct_d = nc.declare_dram_parameter("ct", [L, L], f32, isOutput=False)
    ones_d = nc.declare_dram_parameter("ones1", [1, L], f32, isOutput=False)
    out_d = nc.declare_dram_parameter("out", [128, FB], f32, isOutput=True)

    with tile.TileContext(nc) as tc:
        with (
            tc.tile_pool(name="persist", bufs=1) as pp,
            tc.tile_pool(name="stream", bufs=2) as sp,
            tc.tile_pool(name="epi", bufs=3) as ep,
            tc.tile_pool(name="work", bufs=2) as wp,
            tc.tile_pool(name="dram", bufs=2, space="DRAM") as dp,
            tc.tile_pool(name="ps_bil", bufs=1, space="PSUM") as ps_bil,
        ):
            # ---------------- persistent SBUF ----------------
            sb_kb = pp.tile([128, NTILE * NYX], f16, tag="kb")       # 128KB/p
            sb_kyx = pp.tile([128, NT * NYX], f16, tag="kyx")        # 16KB/p
            sb_unary = pp.tile([128, NC * FB], f32, tag="unary")     # 5.25KB/p
            sb_unown = pp.tile([128, FB], f32, tag="unown")
            sb_s1 = pp.tile([128, NC * FB1], f16, tag="s1")          # 4.1KB/p
            sb_slots = [pp.tile([128, NC * FB], f16, tag=f"slots{j % 2}",
                                name=f"sb_slots{j}")
                        for j in range(NUM_ITER - 1)]                # ping-pong
            sb_featc = pp.tile([6, NYX], f16, tag="featc")
            sb_sqh = pp.tile([128, NTILE], f32, tag="sqh")
            sb_zco = pp.tile([128, NC], f32, tag="zco")
            sb_wst = pp.tile([L, L], f32, tag="wst")
            sb_wbt = pp.tile([L, L], f32, tag="wbt")
            sb_ct = pp.tile([L, L], f32, tag="ct")
            sb_ones = pp.tile([1, L], f32, tag="ones")
            sb_recipb = pp.tile([L, NYX], f32, tag="recipb")
            sb_nrcp = pp.tile([1, NYX], f32, tag="nrcp")
            sb_exp1 = pp.tile([128, NC * FB], f16, tag="exp1")       # iter-1 exp
            sb_red1 = pp.tile([128, NC * NT], f32, tag="red1")
            sb_rcp1 = pp.tile([128, NC * NT], f32, tag="rcp1")
            sb_out = pp.tile([128, FB], f32, tag="outt")

            # ---------------- input DMAs ----------------
            nc.sync.dma_start(sb_featc[:, :], featc_d[:, :])
            nc.sync.dma_start(sb_sqh[:, :], sqh_d[:, :])
            nc.sync.dma_start(sb_kyx[:, :], kyx_d[:, :])
            nc.sync.dma_start(sb_unary[:, :], unary_d[:, :])
            nc.sync.dma_start(sb_unown[:, :], unown_d[:, :])
            nc.sync.dma_start(sb_zco[:, :], zco_d[:, :])
            nc.sync.dma_start(sb_wst[:, :], wst_d[:, :])
            nc.sync.dma_start(sb_wbt[:, :], wbt_d[:, :])
            nc.sync.dma_start(sb_ct[:, :], ct_d[:, :])
            nc.sync.dma_start(sb_ones[:, :], ones_d[:, :])

            # ---------------- iter-1 softmax for all 8 slices ----------------
            nc.scalar.activation(sb_exp1[:, :], sb_unary[:, :], AF.Exp)
            un_v = sb_exp1[:, :].rearrange("p (g l) -> p g l", l=L)      # [128, 64, 21]
            nc.vector.tensor_reduce(sb_red1[:, :], un_v, mybir.AxisListType.X, OP.add)
            nc.vector.reciprocal(sb_rcp1[:, :], sb_red1[:, :])
            nc.vector.memset(sb_s1[:, :], 0.0)
            s1_v = sb_s1[:, :].rearrange("p (g l) -> p g l", l=LW1)      # [128, 64, 33]
            nc.vector.tensor_tensor(
                s1_v[:, :, 0:L], un_v,
                sb_rcp1[:, :].broadcast_to([128, NC * NT, L]),
                OP.mult,
            )
            nc.vector.memset(s1_v[:, :, LW1 - 1:LW1], 1.0)

            # ---------------- K_b build ----------------
            kb_v = sb_kb[:, :].rearrange("p (n c) -> p n c", c=NYX)      # [128, 64, 1024]
            s1_l = sb_s1[:, :].rearrange("p (n l) -> p n l", l=LW1)      # [128, 64, 33]
            with tc.tile_pool(name="ps_g", bufs=3, space="PSUM") as ps_g:
                for mc in range(NT):  # macro chunks of 8 tiles
                    fr = sp.tile([6, NYX], f16, tag="fr")
                    nc.sync.dma_start(fr[:, :], featr_d[:, mc * NYX:(mc + 1) * NYX])
                    for tl in range(NT):
                        dt = mc * NT + tl
                        g = ps_g.tile([128, NYX], f32, tag="g")
                        for h in range(2):
                            nc.tensor.matmul(
                                g[:, h * 512:(h + 1) * 512],
                                fr[:, tl * 128:(tl + 1) * 128],
                                sb_featc[:, h * 512:(h + 1) * 512],
                                start=True, stop=True,
                            )
                        nc.scalar.activation(
                            kb_v[:, dt, :], g[:, :],
                            AF.Exp, bias=sb_sqh[:, dt:dt + 1],
                        )

            with tc.tile_pool(name="ps_rest", bufs=1, space="PSUM") as ps_r:
                # ================= iterations =================
                slots_l = s1_l
                slots_dt = sb_s1[:, :].rearrange(
                    "p (d t l) -> p d t l", d=NC, l=LW1)

                for it in range(NUM_ITER):
                    last = it == NUM_ITER - 1
                    cur_bil = ps_bil.tile([128, NYX], f32, tag="bil")
                    if it == 0:
                        # plain matmuls, M=33 (ones row at partition 32 -> norm)
                        for dt in range(NTILE):
                            for h in range(2):
                                nc.tensor.matmul(
                                    cur_bil[0:LW1, h * 512:(h + 1) * 512],
                                    slots_l[:, dt, :],
                                    kb_v[:, dt, h * 512:(h + 1) * 512],
                                    start=(dt == 0), stop=(dt == NTILE - 1),
                                    skip_group_check=True,
                                )
                        # norm reciprocal + broadcast to 21 rows
                        nc.vector.reciprocal(sb_nrcp[:, :], cur_bil[LW1 - 1:LW1, :])
                        ps_nb = ps_r.tile([L, NYX], f32, tag="spat")
                        for h in range(2):
                            nc.tensor.matmul(
                                ps_nb[:, h * 512:(h + 1) * 512], sb_ones[:, :],
                                sb_nrcp[:, h * 512:(h + 1) * 512],
                                start=True, stop=True,
                            )
                        nc.scalar.copy(sb_recipb[:, :], ps_nb[:, :])
                        mbn = ep.tile([L, NYX], f32, tag="epi")
                        nc.vector.tensor_tensor(
                            mbn[:, :], cur_bil[0:L, :], sb_recipb[:, :], OP.mult)
                    else:
                        # 4x column-tiled: group g computes chunks g*16..g*16+15
                        # into psum partitions 32g..32g+20, then DVE-reduce
                        for j in range(16):
                            for h in range(2):
                                for g in range(4):
                                    dt = g * 16 + j
                                    nc.tensor.matmul(
                                        cur_bil[32 * g:32 * g + L,
                                                h * 512:(h + 1) * 512],
                                        slots_l[:, dt, 0:L],
                                        kb_v[:, dt, h * 512:(h + 1) * 512],
                                        start=(j == 0), stop=(j == 15),
                                        skip_group_check=True,
                                        tile_position=(0, 32 * g),
                                    )
                        a1 = ep.tile([L, NYX], f32, tag="epi")
                        nc.scalar.copy(a1[:, :], cur_bil[32:32 + L, :])
                        a3 = ep.tile([L, NYX], f32, tag="epi")
                        nc.scalar.copy(a3[:, :], cur_bil[96:96 + L, :])
                        t1 = ep.tile([L, NYX], f32, tag="epi")
                        nc.vector.tensor_tensor(
                            t1[:, :], cur_bil[0:L, :], a1[:, :], OP.add)
                        t2 = ep.tile([L, NYX], f32, tag="epi")
                        nc.vector.tensor_tensor(
                            t2[:, :], cur_bil[64:64 + L, :], a3[:, :], OP.add)
                        u = ep.tile([L, NYX], f32, tag="epi")
                        nc.vector.tensor_tensor(u[:, :], t1[:, :], t2[:, :], OP.add)
                        mbn = ep.tile([L, NYX], f32, tag="epi")
                        nc.vector.tensor_tensor(
                            mbn[:, :], u[:, :], sb_recipb[:, :], OP.mult)

                    # ---- spatial: z-mix then yx filter ----
                    brt = wp.tile([128, FB], f16, tag="brt")
                    nc.vector.tensor_scalar_mul(
                        brt[:, :], slots_dt[:, 0, :, 0:L], sb_zco[:, 0:1])
                    for d in range(1, NC):
                        nc.vector.scalar_tensor_tensor(
                            brt[:, :], slots_dt[:, d, :, 0:L], sb_zco[:, d:d + 1],
                            brt[:, :], OP.mult, OP.add)
                    brt_v = brt[:, :].rearrange("p (t l) -> p t l", l=L)
                    ps_sp = ps_r.tile([L, NYX], f32, tag="spat")
                    kyx_v = sb_kyx[:, :].rearrange("p (k c) -> p k c", c=NYX)
                    for k in range(NT):
                        for h in range(2):
                            nc.tensor.matmul(
                                ps_sp[:, h * 512:(h + 1) * 512],
                                brt_v[:, k, :],
                                kyx_v[:, k, h * 512:(h + 1) * 512],
                                start=(k == 0), stop=(k == NT - 1),
                                skip_group_check=True,
                            )
                    msn = ep.tile([L, NYX], f32, tag="epi")
                    nc.scalar.copy(msn[:, :], ps_sp[:, :])

                    # ---- label mixing: comb = Ws@Msn + Wb@Mbn ----
                    ps_cb = ps_r.tile([L, NYX], f32, tag="comb")
                    for h in range(2):
                        nc.tensor.matmul(
                            ps_cb[:, h * 512:(h + 1) * 512], sb_wst[:, :],
                            msn[:, h * 512:(h + 1) * 512],
                            start=True, stop=False, skip_group_check=True)
                        nc.tensor.matmul(
                            ps_cb[:, h * 512:(h + 1) * 512], sb_wbt[:, :],
                            mbn[:, h * 512:(h + 1) * 512],
                            start=False, stop=True, skip_group_check=True)
                    comb = ep.tile([L, NYX], f32, tag="epi")
                    nc.scalar.copy(comb[:, :], ps_cb[:, :])

                    # ---- curT = comb^T @ C^T (+ unary), voxel-major ----
                    ps_ct = ps_r.tile([128, FB], f32, tag="curt")
                    for tl in range(NT):
                        nc.tensor.matmul(
                            ps_ct[:, tl * L:(tl + 1) * L],
                            comb[:, tl * 128:(tl + 1) * 128],
                            sb_ct[:, :], start=True, stop=True)
                    sm = wp.tile([128, FB], f32, tag="sum")
                    nc.vector.tensor_tensor(
                        sm[:, :], ps_ct[:, :], sb_unown[:, :], OP.add)

                    # ---- softmax over labels (free dim) ----
                    ex = wp.tile([128, FB], f32, tag="exp")
                    nc.scalar.activation(ex[:, :], sm[:, :], AF.Exp)
                    ex_v = ex[:, :].rearrange("p (t l) -> p t l", l=L)
                    rd = wp.tile([128, NT], f32, tag="red")
                    nc.vector.tensor_reduce(
                        rd[:, :], ex_v, mybir.AxisListType.X, OP.add)
                    rc = wp.tile([128, NT], f32, tag="rcp")
                    nc.vector.reciprocal(rc[:, :], rd[:, :])
                    if last:
                        nc.vector.tensor_tensor(
                            sb_out[:, :].rearrange("p (t l) -> p t l", l=L),
                            ex_v, rc[:, :].broadcast_to([128, NT, L]), OP.mult)
                        nc.sync.dma_start(out_d[:, :], sb_out[:, :])
                    else:
                        qblk = wp.tile([128, FB], f16, tag="qblk")
                        nc.vector.tensor_tensor(
                            qblk[:, :].rearrange("p (t l) -> p t l", l=L),
                            ex_v, rc[:, :].broadcast_to([128, NT, L]), OP.mult)

                        # ---- exchange: AllGather of the q blocks ----
                        cc_in = dp.tile([128, FB], f16, tag="ccin")
                        cc_out = dp.tile([128 * NC, FB], f16, tag="ccout")
                        nc.sync.dma_start(cc_in[:, :], qblk[:, :])
                        nc.gpsimd.collective_compute(
                            "AllGather",
                            mybir.AluOpType.bypass,
                            replica_groups=[list(range(NC))],
                            ins=[cc_in.opt()],
                            outs=[cc_out.opt()],
                        )
                        nxt = sb_slots[it]
                        nc.sync.dma_start(
                            nxt[:, :].rearrange("p (d f) -> p d f", d=NC),
                            cc_out[:, :].rearrange("(d p) f -> p d f", p=128),
                        )
                        slots_l = nxt[:, :].rearrange("p (n l) -> p n l", l=L)
                        slots_dt = nxt[:, :].rearrange(
                            "p (d t l) -> p d t l", d=NC, l=L)
    nc.compile()
    return nc


def _host_prep(image, logits):
    """Per-core input dicts (global voxel order). Returns list of 8 dicts."""
    img = np.asarray(image, dtype=np.float32)[0]      # [3, D, H, W]
    lg = np.asarray(logits, dtype=np.float32)[0]      # [L, D, H, W]

    zz, yy, xx = np.meshgrid(
        np.arange(D), np.arange(H), np.arange(W), indexing="ij")
    pos = np.stack([zz, yy, xx], -1).reshape(N, 3).astype(np.float32)
    rgb = img.reshape(3, N).T
    feat = np.concatenate([pos / ALPHA, rgb / BETA], axis=1).astype(np.float16)
    featf = feat.astype(np.float32)
    sq = np.sum(featf * featf, axis=1)                # [N], from fp16-rounded

    r1 = np.arange(D, dtype=np.float32)
    Gz = np.exp(-0.5 * ((r1[:, None] - r1[None, :]) / GAMMA) ** 2)
    r2 = np.arange(H, dtype=np.float32)
    Gy = np.exp(-0.5 * ((r2[:, None] - r2[None, :]) / GAMMA) ** 2)
    Kyx = np.kron(Gy, Gy).astype(np.float32)          # H == W so Gy == Gx
    nyx = Kyx.sum(axis=0)
    Kyx_n = (Kyx / nyx[None, :]).astype(np.float16)   # [1024, 1024]
    czsum = Gz.sum(axis=0)

    unary = lg.reshape(L, N)
    # voxel-major: blkT[p, s, t*L + l] = unary[l, s*NYX + t*128 + p]
    blkT = unary.reshape(L, D, NT, 128).transpose(3, 1, 2, 0)  # [128, D, NT, L]
    un = np.ascontiguousarray(blkT.reshape(128, NC * FB))

    featr = np.ascontiguousarray(feat.T)              # [6, N] fp16
    sqh = np.ascontiguousarray((-0.5 * sq).reshape(NTILE, 128).T)  # [128, 64]
    kyx_in = np.ascontiguousarray(
        Kyx_n.reshape(NT, 128, NYX).transpose(1, 0, 2).reshape(128, NT * NYX))

    maps = []
    for r in range(NC):
        featc = np.ascontiguousarray(feat[r * NYX:(r + 1) * NYX].T)
        zco = np.tile((Gz[:, r] / czsum[r]).astype(np.float32), (128, 1))
        unown = np.ascontiguousarray(blkT[:, r].reshape(128, FB))
        maps.append({
            "featr": featr,
            "featc": featc,
            "sqh": sqh,
            "kyx": kyx_in,
            "unaryt": un,
            "unown": unown,
            "zcoef": np.ascontiguousarray(zco),
        })
    return maps


def kernel(image, logits, spatial_ker_weights, bilateral_ker_weights,
           compatibility_matrix):
    from concourse.bass_utils import run_bass_kernel_spmd

    if "nc" not in _CACHE:
        _CACHE["nc"] = _build_nc()
    nc = _CACHE["nc"]

    maps = _host_prep(image, logits)
    wst = np.ascontiguousarray(
        np.asarray(spatial_ker_weights, np.float32).T)
    wbt = np.ascontiguousarray(
        np.asarray(bilateral_ker_weights, np.float32).T)
    ct = np.ascontiguousarray(
        np.asarray(compatibility_matrix, np.float32).T)
    ones1 = np.ones((1, L), np.float32)
    for m in maps:
        m["wst"] = wst
        m["wbt"] = wbt
        m["ct"] = ct
        m["ones1"] = ones1

    res = run_bass_kernel_spmd(nc, maps, core_ids=list(range(NC)))

    out = np.empty((L, D, H, W), dtype=np.float32)
    for r in range(NC):
        blk = res.results[r]["out"]                   # [128, 168]
        out[:, r] = blk.reshape(128, NT, L).transpose(2, 1, 0).reshape(L, H, W)
    return out[None]


# revision 10
# speedup vs baseline: 28.1073x; 1.1834x over previous
"""CRF-RNN (dense Gaussian CRF mean-field) Trainium2 kernel, 8 NeuronCores.

Strategy
--------
N = 8*32*32 = 8192 voxels, L = 21 labels, 5 mean-field iterations.
- Column sharding: core r owns z-slice r (1024 voxels = its output columns).
- Bilateral kernel: each core builds U[:, own] = exp(feat.feat^T - |feat_r|^2/2)
  on device (fp16, SBUF-resident, 16MB). The per-column factor exp(-|feat_c|^2/2)
  cancels against the normalization, so it is never materialized. The
  normalizer is obtained for free as an extra "ones" row in the iter-1 matmul
  (M=22 still fits a 32-wide PE column-tile group).
- Spatial kernel is exactly separable (grid tensor product):
  q@K_s[:, cols_r] = (sum_z Gz[z,r] q[:,z]) @ (Gy x Gx). The z-mix commutes with
  the yx-filter; norm_s is folded into Kyx columns + the z-mix coefficients.
- Per-iteration cross-core exchange of each core's q block ([128,168] fp16)
  via AllGather through DRAM bounce buffers (ncfw collective).
- Softmax runs voxel-major (labels on the free dim) - no cross-partition ops.
"""

import numpy as np

ALPHA, BETA, GAMMA = 160.0, 3.0, 3.0
NUM_ITER = 5
L, D, H, W = 21, 8, 32, 32
NC = 8
NYX = H * W            # 1024
N = D * NYX            # 8192
NT = NYX // 128        # 8 chunks per slice
FB = NT * L            # 168  free width of one q block
LW1 = 22               # iter-1 slot entry width: labels 0..20, ones column at 21
FB1 = NT * LW1
NTILE = N // 128       # 64 row tiles of the bilateral kernel

_CACHE = {}


def _build_nc():
    import concourse.bass as bass
    import concourse.bacc as bacc
    import concourse.mybir as mybir
    import concourse.tile as tile
    import concourse.tile_utils as tile_utils

    # cayman has 208KB/partition usable; the default cap is stale at 192KB
    try:
        tile_utils.max_sbuf_usage = 204 * 1024
    except Exception:
        pass

    f32 = mybir.dt.float32
    f16 = mybir.dt.float16
    AF = mybir.ActivationFunctionType
    OP = mybir.AluOpType

    nc = bacc.Bacc(None, target_bir_lowering=False, num_devices=NC)

    # ---- DRAM I/O (fp16 features; global voxel order everywhere) ----
    featr_d = nc.declare_dram_parameter("featr", [6, N], f16, isOutput=False)
    featc_d = nc.declare_dram_parameter("featc", [6, NYX], f16, isOutput=False)
    sqh_d = nc.declare_dram_parameter("sqh", [128, NTILE], f32, isOutput=False)
    kyx_d = nc.declare_dram_parameter("kyx", [128, NT * NYX], f16, isOutput=False)
    unary_d = nc.declare_dram_parameter("unaryt", [128, NC * FB], f32, isOutput=False)
    unown_d = nc.declare_dram_parameter("unown", [128, FB], f32, isOutput=False)
    zco_d = nc.declare_dram_parameter("zcoef", [128, NC], f32, isOutput=False)
    wst_d = nc.declare_dram_parameter("wst", [L, L], f32, isOutput=False)
    wbt_d = nc.declare_dram_parameter("wbt", [L, L], f32, isOutput=False)
    ct_d = nc.declare_dram_parameter("ct", [L, L], f32, isOutput=False)
    ones_d = nc.declare_dram_parameter("ones1", [1, L], f32, isOutput=False)
    out_d = nc.declare_dram_parameter("out", [128, FB], f32, isOutput=True)

    with tile.TileContext(nc) as tc:
        with (
            tc.tile_pool(name="persist", bufs=1) as pp,
            tc.tile_pool(name="stream", bufs=2) as sp,
            tc.tile_pool(name="epi", bufs=3) as ep,
            tc.tile_pool(name="work", bufs=2) as wp,
            tc.tile_pool(name="dram", bufs=2, space="DRAM") as dp,
            tc.tile_pool(name="ps_bil", bufs=1, space="PSUM") as ps_bil,
        ):
            # ---------------- persistent SBUF ----------------
            sb_kb = pp.tile([128, NTILE * NYX], f16, tag="kb")       # 128KB/p
            sb_kyx = pp.tile([128, NT * NYX], f16, tag="kyx")        # 16KB/p
            sb_unary = pp.tile([128, NC * FB], f32, tag="unary")     # 5.25KB/p
            sb_unown = pp.tile([128, FB], f32, tag="unown")
            sb_s1 = pp.tile([128, NC * FB1], f16, tag="s1")          # 4.1KB/p
            sb_slots = [pp.tile([128, NC * FB], f16, tag=f"slots{j % 2}",
                                name=f"sb_slots{j}")
                        for j in range(NUM_ITER - 1)]                # ping-pong
            sb_featc = pp.tile([6, NYX], f16, tag="featc")
            sb_sqh = pp.tile([128, NTILE], f32, tag="sqh")
            sb_zco = pp.tile([128, NC], f32, tag="zco")
            sb_wst = pp.tile([L, L], f32, tag="wst")
            sb_wbt = pp.tile([L, L], f32, tag="wbt")
            sb_ct = pp.tile([L, L], f32, tag="ct")
            sb_ones = pp.tile([1, L], f32, tag="ones")
            sb_recipb = pp.tile([L, NYX], f32, tag="recipb")
            sb_nrow = pp.tile([1, NYX], f32, tag="nrow")
            sb_nrcp = pp.tile([1, NYX], f32, tag="nrcp")
            sb_exp1 = pp.tile([128, NC * FB], f16, tag="exp1")       # iter-1 exp
            sb_red1 = pp.tile([128, NC * NT], f32, tag="red1")
            sb_rcp1 = pp.tile([128, NC * NT], f32, tag="rcp1")
            sb_out = pp.tile([128, FB], f32, tag="outt")

            # ---------------- input DMAs ----------------
            nc.sync.dma_start(sb_featc[:, :], featc_d[:, :])
            nc.sync.dma_start(sb_sqh[:, :], sqh_d[:, :])
            nc.sync.dma_start(sb_kyx[:, :], kyx_d[:, :])
            nc.sync.dma_start(sb_unary[:, :], unary_d[:, :])
            nc.sync.dma_start(sb_unown[:, :], unown_d[:, :])
            nc.sync.dma_start(sb_zco[:, :], zco_d[:, :])
            nc.sync.dma_start(sb_wst[:, :], wst_d[:, :])
            nc.sync.dma_start(sb_wbt[:, :], wbt_d[:, :])
            nc.sync.dma_start(sb_ct[:, :], ct_d[:, :])
            nc.sync.dma_start(sb_ones[:, :], ones_d[:, :])

            # warm up the collective path early (overlaps the K_b build)
            wu_in = dp.tile([128, 8], f16, tag="wuin")
            wu_out = dp.tile([128 * NC, 8], f16, tag="wuout")
            wu_sb = pp.tile([128, 8], f16, tag="wusb")
            nc.vector.memset(wu_sb[:, :], 0.0)
            nc.sync.dma_start(wu_in[:, :], wu_sb[:, :])
            nc.gpsimd.collective_compute(
                "AllGather", mybir.AluOpType.bypass,
                replica_groups=[list(range(NC))],
                ins=[wu_in.opt()], outs=[wu_out.opt()],
            )

            # ---------------- iter-1 softmax for all 8 slices ----------------
            nc.scalar.activation(sb_exp1[:, :], sb_unary[:, :], AF.Exp)
            un_v = sb_exp1[:, :].rearrange("p (g l) -> p g l", l=L)      # [128, 64, 21]
            nc.vector.tensor_reduce(sb_red1[:, :], un_v, mybir.AxisListType.X, OP.add)
            nc.vector.reciprocal(sb_rcp1[:, :], sb_red1[:, :])
            nc.vector.memset(sb_s1[:, :], 0.0)
            s1_v = sb_s1[:, :].rearrange("p (g l) -> p g l", l=LW1)      # [128, 64, 33]
            nc.vector.tensor_tensor(
                s1_v[:, :, 0:L], un_v,
                sb_rcp1[:, :].broadcast_to([128, NC * NT, L]),
                OP.mult,
            )
            nc.vector.memset(s1_v[:, :, LW1 - 1:LW1], 1.0)

            # ---------------- K_b build ----------------
            kb_v = sb_kb[:, :].rearrange("p (n c) -> p n c", c=NYX)      # [128, 64, 1024]
            s1_l = sb_s1[:, :].rearrange("p (n l) -> p n l", l=LW1)      # [128, 64, 33]
            with tc.tile_pool(name="ps_g", bufs=3, space="PSUM") as ps_g:
                for mc in range(NT):  # macro chunks of 8 tiles
                    fr = sp.tile([6, NYX], f16, tag="fr")
                    nc.sync.dma_start(fr[:, :], featr_d[:, mc * NYX:(mc + 1) * NYX])
                    for tl in range(NT):
                        dt = mc * NT + tl
                        g = ps_g.tile([128, NYX], f32, tag="g")
                        for h in range(2):
                            nc.tensor.matmul(
                                g[:, h * 512:(h + 1) * 512],
                                fr[:, tl * 128:(tl + 1) * 128],
                                sb_featc[:, h * 512:(h + 1) * 512],
                                start=True, stop=True,
                            )
                        nc.scalar.activation(
                            kb_v[:, dt, :], g[:, :],
                            AF.Exp, bias=sb_sqh[:, dt:dt + 1],
                        )

            with tc.tile_pool(name="ps_rest", bufs=1, space="PSUM") as ps_r:
                # ================= iterations =================
                slots_l = s1_l
                slots_dt = sb_s1[:, :].rearrange(
                    "p (d t l) -> p d t l", d=NC, l=LW1)

                for it in range(NUM_ITER):
                    last = it == NUM_ITER - 1
                    rows = LW1 if it == 0 else L   # iter 0 carries the ones row
                    cur_bil = ps_bil.tile([128, NYX], f32, tag="bil")
                    # 4x column-tiled bilateral; group g handles chunks
                    # dt = 4j+g so all groups become ready in lockstep with
                    # the K_b build's exp stream (iter 0)
                    for j in range(16):
                        for h in range(2):
                            for g in range(4):
                                dt = 4 * j + g
                                nc.tensor.matmul(
                                    cur_bil[32 * g:32 * g + rows,
                                            h * 512:(h + 1) * 512],
                                    slots_l[:, dt, 0:rows],
                                    kb_v[:, dt, h * 512:(h + 1) * 512],
                                    start=(j == 0), stop=(j == 15),
                                    skip_group_check=True,
                                    tile_position=(0, 32 * g),
                                )
                    a1 = ep.tile([rows, NYX], f32, tag="epi")
                    nc.scalar.copy(a1[:, :], cur_bil[32:32 + rows, :])
                    a3 = ep.tile([rows, NYX], f32, tag="epi")
                    nc.scalar.copy(a3[:, :], cur_bil[96:96 + rows, :])
                    t1 = ep.tile([rows, NYX], f32, tag="epi")
                    nc.vector.tensor_tensor(
                        t1[:, :], cur_bil[0:rows, :], a1[:, :], OP.add)
                    t2 = ep.tile([rows, NYX], f32, tag="epi")
                    nc.vector.tensor_tensor(
                        t2[:, :], cur_bil[64:64 + rows, :], a3[:, :], OP.add)
                    u = ep.tile([rows, NYX], f32, tag="epi")
                    nc.vector.tensor_tensor(u[:, :], t1[:, :], t2[:, :], OP.add)
                    if it == 0:
                        # norm row -> partition 0 (DMA: engines can't address
                        # partition base 21), reciprocal, broadcast to 21 rows
                        nc.sync.dma_start(sb_nrow[:, :], u[L:LW1, :])
                        nc.vector.reciprocal(sb_nrcp[:, :], sb_nrow[:, :])
                        ps_nb = ps_r.tile([L, NYX], f32, tag="spat")
                        for h in range(2):
                            nc.tensor.matmul(
                                ps_nb[:, h * 512:(h + 1) * 512], sb_ones[:, :],
                                sb_nrcp[:, h * 512:(h + 1) * 512],
                                start=True, stop=True,
                            )
                        nc.scalar.copy(sb_recipb[:, :], ps_nb[:, :])
                    mbn = ep.tile([L, NYX], f32, tag="epi")
                    nc.vector.tensor_tensor(
                        mbn[:, :], u[0:L, :], sb_recipb[:, :], OP.mult)

                    # ---- spatial: z-mix then yx filter ----
                    brt = wp.tile([128, FB], f16, tag="brt")
                    nc.vector.tensor_scalar_mul(
                        brt[:, :], slots_dt[:, 0, :, 0:L], sb_zco[:, 0:1])
                    for d in range(1, NC):
                        nc.vector.scalar_tensor_tensor(
                            brt[:, :], slots_dt[:, d, :, 0:L], sb_zco[:, d:d + 1],
                            brt[:, :], OP.mult, OP.add)
                    brt_v = brt[:, :].rearrange("p (t l) -> p t l", l=L)
                    ps_sp = ps_r.tile([L, NYX], f32, tag="spat")
                    kyx_v = sb_kyx[:, :].rearrange("p (k c) -> p k c", c=NYX)
                    for k in range(NT):
                        for h in range(2):
                            nc.tensor.matmul(
                                ps_sp[:, h * 512:(h + 1) * 512],
                                brt_v[:, k, :],
                                kyx_v[:, k, h * 512:(h + 1) * 512],
                                start=(k == 0), stop=(k == NT - 1),
                                skip_group_check=True,
                            )
                    msn = ep.tile([L, NYX], f32, tag="epi")
                    nc.scalar.copy(msn[:, :], ps_sp[:, :])

                    # ---- label mixing: comb = Ws@Msn + Wb@Mbn ----
                    ps_cb = ps_r.tile([L, NYX], f32, tag="comb")
                    for h in range(2):
                        nc.tensor.matmul(
                            ps_cb[:, h * 512:(h + 1) * 512], sb_wst[:, :],
                            msn[:, h * 512:(h + 1) * 512],
                            start=True, stop=False, skip_group_check=True)
                        nc.tensor.matmul(
                            ps_cb[:, h * 512:(h + 1) * 512], sb_wbt[:, :],
                            mbn[:, h * 512:(h + 1) * 512],
                            start=False, stop=True, skip_group_check=True)
                    comb = ep.tile([L, NYX], f32, tag="epi")
                    nc.scalar.copy(comb[:, :], ps_cb[:, :])

                    # ---- curT = comb^T @ C^T (+ unary), voxel-major ----
                    ps_ct = ps_r.tile([128, FB], f32, tag="curt")
                    for tl in range(NT):
                        nc.tensor.matmul(
                            ps_ct[:, tl * L:(tl + 1) * L],
                            comb[:, tl * 128:(tl + 1) * 128],
                            sb_ct[:, :], start=True, stop=True)
                    sm = wp.tile([128, FB], f32, tag="sum")
                    nc.vector.tensor_tensor(
                        sm[:, :], ps_ct[:, :], sb_unown[:, :], OP.add)

                    # ---- softmax over labels (free dim) ----
                    ex = wp.tile([128, FB], f32, tag="exp")
                    nc.scalar.activation(ex[:, :], sm[:, :], AF.Exp)
                    ex_v = ex[:, :].rearrange("p (t l) -> p t l", l=L)
                    rd = wp.tile([128, NT], f32, tag="red")
                    nc.vector.tensor_reduce(
                        rd[:, :], ex_v, mybir.AxisListType.X, OP.add)
                    rc = wp.tile([128, NT], f32, tag="rcp")
                    nc.vector.reciprocal(rc[:, :], rd[:, :])
                    if last:
                        nc.vector.tensor_tensor(
                            sb_out[:, :].rearrange("p (t l) -> p t l", l=L),
                            ex_v, rc[:, :].broadcast_to([128, NT, L]), OP.mult)
                        nc.sync.dma_start(out_d[:, :], sb_out[:, :])
                    else:
                        qblk = wp.tile([128, FB], f16, tag="qblk")
                        nc.vector.tensor_tensor(
                            qblk[:, :].rearrange("p (t l) -> p t l", l=L),
                            ex_v, rc[:, :].broadcast_to([128, NT, L]), OP.mult)

                        # ---- exchange: AllGather of the q blocks ----
                        cc_in = dp.tile([128, FB], f16, tag="ccin")
                        cc_out = dp.tile([128 * NC, FB], f16, tag="ccout")
                        nc.sync.dma_start(cc_in[:, :], qblk[:, :])
                        nc.gpsimd.collective_compute(
                            "AllGather",
                            mybir.AluOpType.bypass,
                            replica_groups=[list(range(NC))],
                            ins=[cc_in.opt()],
                            outs=[cc_out.opt()],
                        )
                        nxt = sb_slots[it]
                        nc.sync.dma_start(
                            nxt[:, :].rearrange("p (d f) -> p d f", d=NC),
                            cc_out[:, :].rearrange("(d p) f -> p d f", p=128),
                        )
                        slots_l = nxt[:, :].rearrange("p (n l) -> p n l", l=L)
                        slots_dt = nxt[:, :].rearrange(
                            "p (d t l) -> p d t l", d=NC, l=L)
    nc.compile()
    return nc


def _host_prep(image, logits):
    """Per-core input dicts (global voxel order). Returns list of 8 dicts."""
    img = np.asarray(image, dtype=np.float32)[0]      # [3, D, H, W]
    lg = np.asarray(logits, dtype=np.float32)[0]      # [L, D, H, W]

    zz, yy, xx = np.meshgrid(
        np.arange(D), np.arange(H), np.arange(W), indexing="ij")
    pos = np.stack([zz, yy, xx], -1).reshape(N, 3).astype(np.float32)
    rgb = img.reshape(3, N).T
    feat = np.concatenate([pos / ALPHA, rgb / BETA], axis=1).astype(np.float16)
    featf = feat.astype(np.float32)
    sq = np.sum(featf * featf, axis=1)                # [N], from fp16-rounded

    r1 = np.arange(D, dtype=np.float32)
    Gz = np.exp(-0.5 * ((r1[:, None] - r1[None, :]) / GAMMA) ** 2)
    r2 = np.arange(H, dtype=np.float32)
    Gy = np.exp(-0.5 * ((r2[:, None] - r2[None, :]) / GAMMA) ** 2)
    Kyx = np.kron(Gy, Gy).astype(np.float32)          # H == W so Gy == Gx
    nyx = Kyx.sum(axis=0)
    Kyx_n = (Kyx / nyx[None, :]).astype(np.float16)   # [1024, 1024]
    czsum = Gz.sum(axis=0)

    unary = lg.reshape(L, N)
    # voxel-major: blkT[p, s, t*L + l] = unary[l, s*NYX + t*128 + p]
    blkT = unary.reshape(L, D, NT, 128).transpose(3, 1, 2, 0)  # [128, D, NT, L]
    un = np.ascontiguousarray(blkT.reshape(128, NC * FB))

    featr = np.ascontiguousarray(feat.T)              # [6, N] fp16
    sqh = np.ascontiguousarray((-0.5 * sq).reshape(NTILE, 128).T)  # [128, 64]
    kyx_in = np.ascontiguousarray(
        Kyx_n.reshape(NT, 128, NYX).transpose(1, 0, 2).reshape(128, NT * NYX))

    maps = []
    for r in range(NC):
        featc = np.ascontiguousarray(feat[r * NYX:(r + 1) * NYX].T)
        zco = np.tile((Gz[:, r] / czsum[r]).astype(np.float32), (128, 1))
        unown = np.ascontiguousarray(blkT[:, r].reshape(128, FB))
        maps.append({
            "featr": featr,
            "featc": featc,
            "sqh": sqh,
            "kyx": kyx_in,
            "unaryt": un,
            "unown": unown,
            "zcoef": np.ascontiguousarray(zco),
        })
    return maps


def kernel(image, logits, spatial_ker_weights, bilateral_ker_weights,
           compatibility_matrix):
    from concourse.bass_utils import run_bass_kernel_spmd

    if "nc" not in _CACHE:
        _CACHE["nc"] = _build_nc()
    nc = _CACHE["nc"]

    maps = _host_prep(image, logits)
    wst = np.ascontiguousarray(
        np.asarray(spatial_ker_weights, np.float32).T)
    wbt = np.ascontiguousarray(
        np.asarray(bilateral_ker_weights, np.float32).T)
    ct = np.ascontiguousarray(
        np.asarray(compatibility_matrix, np.float32).T)
    ones1 = np.ones((1, L), np.float32)
    for m in maps:
        m["wst"] = wst
        m["wbt"] = wbt
        m["ct"] = ct
        m["ones1"] = ones1

    res = run_bass_kernel_spmd(nc, maps, core_ids=list(range(NC)))

    out = np.empty((L, D, H, W), dtype=np.float32)
    for r in range(NC):
        blk = res.results[r]["out"]                   # [128, 168]
        out[:, r] = blk.reshape(128, NT, L).transpose(2, 1, 0).reshape(L, H, W)
    return out[None]
